# revision 19
# baseline (speedup 1.0000x reference)
"""Causal flash attention for Trainium2, sharded 2 heads/core over 8 cores.

Math per head: out = softmax_causal(Q K^T / sqrt(D)) @ V,  Q/K/V [S=2048, D=64] fp32.

Device layout (per core, heads h0=2c, h1=2c+1):
  qT   [128, 2048]  rows 64h+d = Q[h]^T        (D on partitions, both heads stacked)
  kT   [128, 2048]  same for K
  val  [2, 128, 1024] val[h, p, 64*kc+d] = V[h, 128*kc+p, d]
  outT [128, 2048]  rows 64h+d = out[h]^T (normalized)

Scores are computed transposed (S^T[k, q] = K_chunk @ Q^T) so no transposes are
needed anywhere.

exp is the serial bottleneck (one elem/cycle/lane on the scalar engine), so the
key chunks are split between two engines: the scalar engine computes exact exp,
and the vector engine computes a Schraudolph-style approximation (y = A*x + B
converted to int16, bitcast as bf16 = 2^(x*log2e) with mantissa-linear
interpolation, ~2% rms which softmax normalization partly cancels); the DVE
share of elements is capped to keep the end-to-end error ~1e-2.

PE work per chunk is minimized with PE array tiling:
  - QK: the two heads' matmuls occupy disjoint row halves (contraction D=64
    each) and run concurrently.
  - PV: head outputs go to col groups 0:64 / 64:128 of ONE psum bank
    (tile_position=(0,64) for h1) and run concurrently.
  - denominators: ones-vector matmuls into single psum rows {0,32,64,96} =
    (head, kc parity) of a shared D bank; a chunk pair's four matmuls occupy
    four distinct col groups and run concurrently.
po and D are single psum banks; PV/denom emission for the first chunks of a
span is deferred so the previous span's tail (which reads po/D) never
head-of-line blocks the PE FIFO.
"""

import os
import sys

import ml_dtypes
import numpy as np

sys.path.insert(0, "/opt/trn_rl_repo")

import concourse.bass as bass
import concourse.bacc as bacc
import concourse.mybir as mybir
import concourse.tile as tile
from concourse.bass_utils import run_bass_kernel_spmd

B, H, S, D = 1, 16, 2048, 64
N_CORES = 8
HEADS_PER_CORE = H // N_CORES  # 2
N_CHUNKS = S // 128  # 16 key chunks per head
N_SPANS = S // 512  # 4 query spans per head
F32 = mybir.dt.float32
R32 = mybir.dt.float32r
BF16 = mybir.dt.bfloat16
F16 = mybir.dt.float16
I16 = mybir.dt.int16

# Schraudolph constants for bf16-bit exp of (0.125 * x):
#   i16 = 0.125*x*128*log2(e) + (127*128 - C) ; bitcast bf16 ~= exp(0.125*x)
L2E = 1.4426950408889634
SCHRA_A = 0.125 * 128.0 * L2E
SCHRA_B = float(127 * 128 - 3)
DVE_EXP_SHARE = 0.0  # cap on the fraction of score elements exp'd approximately

J_DEFER = 6  # chunks whose PV/denom emission is deferred past the tail window


def _r(ap):
    return ap.bitcast(R32)

_NC = None
_LAST_RESULTS = None


def _build_bass():
    nc = bacc.Bacc("TRN2", target_bir_lowering=False)
    qT = nc.declare_dram_parameter("qT", [128, S], F16, isOutput=False)
    kT = nc.declare_dram_parameter("kT", [128, S], F16, isOutput=False)
    val = nc.declare_dram_parameter("val", [2, 128, 64 * N_CHUNKS], BF16, isOutput=False)
    outT = nc.declare_dram_parameter("outT", [128, S], F32, isOutput=True)

    total_elems = 0
    for s in range(N_SPANS):
        for kc in range(4 * s + 4):
            w = 512 * (s + 1) - max(512 * s, 128 * kc)
            total_elems += 512 + w
    dve_budget = DVE_EXP_SHARE * total_elems

    with tile.TileContext(nc) as tc:
        with (
            tc.tile_pool(name="const", bufs=1) as const,
            tc.tile_pool(name="inbuf", bufs=1) as inbuf,
            tc.tile_pool(name="pbuf", bufs=18) as pbuf,
            tc.tile_pool(name="nbuf", bufs=2) as nbuf,
            tc.tile_pool(name="ps_s", bufs=3, space="PSUM") as ps_s,
            tc.tile_pool(name="ps_po", bufs=1, space="PSUM") as ps_po,
            tc.tile_pool(name="ps_d", bufs=1, space="PSUM") as ps_d,
        ):
            ones1 = const.tile([128, 1], BF16, tag="ones1")
            nc.vector.memset(ones1, 1.0)

            # Input loads, chunked by 512 query/key columns so compute can
            # start before all DMAs land. First span processed is s=3.
            qsb = [None] * N_SPANS
            ksb = [None] * N_SPANS
            vsb = [[None] * N_SPANS, [None] * N_SPANS]
            qsb[3] = inbuf.tile([128, 512], F16, tag="q3", name="qt3")
            nc.sync.dma_start(out=qsb[3], in_=qT[:, 1536:2048])
            for j in range(N_SPANS):
                kt = inbuf.tile([128, 512], F16, tag=f"k{j}", name=f"kt{j}")
                nc.sync.dma_start(out=kt, in_=kT[:, 512 * j : 512 * (j + 1)])
                ksb[j] = kt
                for h in range(2):
                    vt = inbuf.tile([128, 256], BF16, tag=f"v{h}{j}", name=f"vt{h}{j}")
                    nc.sync.dma_start(out=vt, in_=val[h, :, 256 * j : 256 * (j + 1)])
                    vsb[h][j] = vt
            for j in (2, 1, 0):
                qt = inbuf.tile([128, 512], F16, tag=f"q{j}", name=f"qt{j}")
                nc.sync.dma_start(out=qt, in_=qT[:, 512 * j : 512 * (j + 1)])
                qsb[j] = qt

            def k_slice(h, kc):
                return ksb[kc // 4][64 * h : 64 * h + 64, 128 * (kc % 4) : 128 * (kc % 4) + 128]

            def q_slice(h, qs, qe):
                j = qs // 512
                base = 512 * j
                return qsb[j][64 * h : 64 * h + 64, qs - base : qe - base]

            def v_slice(h, kc):
                return vsb[h][kc // 4][:, 64 * (kc % 4) : 64 * (kc % 4) + 64]

            eng_t = {"act": 0.0, "dve": 0.0, "dve_elems": 0.0}

            def emit_pv(h, kc, qs, qe, po, pe2, pv_flags):
                # col-tiled PV: h0 -> partitions 0:64, h1 -> 64:128 of the
                # same bank; the two matmuls run concurrently.
                qb = max(qs, 128 * kc)
                w = qe - qb
                # start/stop are per psum col-group region: h0 and h1 write
                # disjoint partition ranges, each needs its own has_written
                # clear (and the first emitted per head must be full-width).
                start = pv_flags[h] == 0
                stop = pv_flags[h] == pv_flags["total"] // 2 - 1
                pv_flags[h] += 1
                if start:
                    assert w == 512, f"first PV of head {h} must be full-width"
                pv_flags["n"] += 1
                nc.tensor.matmul(
                    po[64 * h : 64 * h + 64, qb - qs : qb - qs + w],
                    v_slice(h, kc),
                    pe2[:, 512 * h : 512 * h + w],
                    start=start,
                    stop=stop,
                    tile_position=(0, 64 * h),
                )

            def emit_den(h, kc, qs, qe, pd, pe2, den_flags, s):
                # denominator partial: ones-vector matmul into psum row
                # 64*h + 32*(kc&1); a chunk pair's four matmuls use four
                # distinct col groups and run concurrently. Span 0 has no
                # full-width chunk of odd parity, so it uses one group per
                # head (first emission must be full-width: start=True only
                # clears has_written for the region it writes).
                qb = max(qs, 128 * kc)
                w = qe - qb
                g = 64 * h + (32 * (kc & 1) if s > 0 else 0)
                start = den_flags[g] == 0
                stop = den_flags[g] == den_flags[g, "total"] - 1
                den_flags[g] += 1
                if start:
                    assert w == 512, f"first den of group {g} must be full (s={s} kc={kc})"
                nc.tensor.matmul(
                    pd[g : g + 1, qb - qs : qb - qs + w],
                    ones1,
                    pe2[:, 512 * h : 512 * h + w],
                    start=start,
                    stop=stop,
                    tile_position=(0, g),
                )

            class Flags(dict):
                def __missing__(self, k):
                    return 0

            def emit_chunks(s, po, pd, deferred):
                qs, qe = 512 * s, 512 * (s + 1)
                nkc = 4 * s + 4
                # defer PV/den emission for the first J chunks (tail WAR
                # window); the first emitted PV/den must be a full-width
                # chunk, so J never exceeds 4s (s=0: everything deferred,
                # flush starts at kc=0 which is full-width).
                J = nkc if s == 0 else min(J_DEFER, 4 * s)
                pe2s = []
                pv_flags = Flags()
                pv_flags["total"] = 2 * nkc
                den_flags = Flags()
                for h in range(2):
                    if s == 0:
                        den_flags[64 * h, "total"] = nkc
                    else:
                        for par in range(2):
                            den_flags[64 * h + 32 * par, "total"] = sum(
                                1 for kc in range(nkc) if (kc & 1) == par
                            )
                den_done = set()
                for kc in range(nkc):
                    # pop the previous span's tail pieces a few chunks in so
                    # their long-latency chains never head-of-line block.
                    if kc in (0, 2, 5) and deferred:
                        deferred.pop(0)()
                    qb = max(qs, 128 * kc)
                    w = qe - qb
                    pg = ps_s.tile([128, 1024], F32, tag="pss", name=f"pg_{s}_{kc}")
                    pe2 = pbuf.tile([128, 1024], BF16, tag="pe", name=f"pe_{s}_{kc}")
                    pe2s.append(pe2)
                    for h in range(2):
                        nc.tensor.matmul(
                            pg[:, 512 * h : 512 * h + w],
                            k_slice(h, kc),
                            q_slice(h, qb, qe),
                            start=True,
                            stop=True,
                        )
                    fd = 512 + w
                    t_act = (fd + 204) / 1.2 + 232
                    t_dve = (fd + 120) / 0.96 + 275
                    use_dve = (
                        eng_t["dve"] + t_dve <= eng_t["act"] + t_act
                        and eng_t["dve_elems"] + fd <= dve_budget
                    )
                    if use_dve:
                        eng_t["dve"] += t_dve
                        eng_t["dve_elems"] += fd
                        nc.vector.tensor_scalar(
                            out=pe2.bitcast(I16)[:, :fd],
                            in0=pg[:, :fd],
                            scalar1=SCHRA_A,
                            scalar2=SCHRA_B,
                            op0=mybir.AluOpType.mult,
                            op1=mybir.AluOpType.add,
                        )
                    else:
                        eng_t["act"] += t_act
                        nc.scalar.activation(
                            out=pe2[:, :fd],
                            in_=pg[:, :fd],
                            func=mybir.ActivationFunctionType.Exp,
                            scale=0.125,
                        )
                    if kc >= 4 * s:
                        # diagonal chunk: zero the strict upper triangle of
                        # the first 128 cols, after exp, on gpsimd.
                        for h in range(2):
                            nc.gpsimd.affine_select(
                                out=pe2[:, 512 * h : 512 * h + 128],
                                in_=pe2[:, 512 * h : 512 * h + 128],
                                compare_op=mybir.AluOpType.is_ge,
                                fill=0.0,
                                base=0,
                                pattern=[[1, 128]],
                                channel_multiplier=-1,
                            )
                    if kc >= J:
                        for h in range(2):
                            emit_pv(h, kc, qs, qe, po, pe2, pv_flags)
                        # in-loop dens only for full-width pairs (both >= J)
                        if kc & 1 and kc < 4 * s and kc - 1 >= J:
                            for cc in (kc - 1, kc):
                                den_done.add(cc)
                                for h in range(2):
                                    emit_den(h, cc, qs, qe, pd, pe2s[cc], den_flags, s)
                # deferred PV matmuls for the first J chunks, then leftover
                # dens, full-width chunks first (psum accumulation is
                # commutative; flags track emission order).
                for kc in range(min(J, nkc)):
                    for h in range(2):
                        emit_pv(h, kc, qs, qe, po, pe2s[kc], pv_flags)
                rest = [kc for kc in range(nkc) if kc not in den_done]
                rest.sort(key=lambda kc: max(qs, 128 * kc))  # full-width first
                for kc in rest:
                    for h in range(2):
                        emit_den(h, kc, qs, qe, pd, pe2s[kc], den_flags, s)
                assert pv_flags["n"] == pv_flags["total"]

            def emit_tail(s, po, pd, part):
                qs, qe = 512 * s, 512 * (s + 1)
                if part == 0:
                    # D bank -> SBUF (one FD-bound DVE copy covers all rows)
                    den_sb = nbuf.tile([97, 512], F32, tag="den")
                    nc.vector.tensor_copy(out=den_sb, in_=pd[0:97, :])
                    # gather the partial rows onto 128 partitions so
                    # reciprocal runs at FD=4 per group; parallel queues.
                    dsum = nbuf.tile([128, 16], F32, tag="dsum")
                    groups = (0, 32, 64, 96) if s > 0 else (0, 64)
                    qq = [nc.sync, nc.gpsimd, nc.sync, nc.gpsimd]
                    for i, g in enumerate(groups):
                        qq[i % len(qq)].dma_start(
                            out=dsum[:, 4 * i : 4 * i + 4],
                            in_=den_sb[g : g + 1, :],
                        )
                    state["dsum"] = dsum
                elif part == 1:
                    dsum = state["dsum"]
                    d8 = nbuf.tile([128, 8], F32, tag="d8")
                    if s > 0:
                        nc.vector.tensor_add(
                            out=d8[:, 0:4], in0=dsum[:, 0:4], in1=dsum[:, 4:8]
                        )
                        nc.vector.tensor_add(
                            out=d8[:, 4:8], in0=dsum[:, 8:12], in1=dsum[:, 12:16]
                        )
                    else:
                        nc.vector.tensor_copy(out=d8, in_=dsum[:, 0:8])
                    r8 = nbuf.tile([128, 8], F32, tag="r8")
                    nc.vector.reciprocal(out=r8, in_=d8)
                    r2 = nbuf.tile([2, 512], F32, tag="r2")
                    for h in range(2):
                        (nc.sync if h == 0 else nc.gpsimd).dma_start(
                            out=r2[h : h + 1, :],
                            in_=r8[:, 4 * h : 4 * h + 4],
                        )
                    rb = nbuf.tile([128, 512], F32, tag="rb")
                    for h in range(2):
                        (nc.sync if h == 0 else nc.gpsimd).dma_start(
                            out=rb[64 * h : 64 * h + 64, :],
                            in_=r2[h : h + 1, :].unsqueeze(1).broadcast_to(
                                [1, 64, 512]
                            ),
                        )
                    state["rb"] = rb
                else:
                    rb = state["rb"]
                    o2 = nbuf.tile([128, 512], F32, tag="o2")
                    nc.vector.tensor_mul(out=o2, in0=po[0:128, :], in1=rb)
                    eng_t["dve"] += 890
                    nc.sync.dma_start(out=outT[:, qs:qe], in_=o2)

            state = {}
            deferred = []
            for s in (3, 2, 1, 0):
                po = ps_po.tile([128, 512], F32, tag="po", name=f"po_{s}")
                pd = ps_d.tile([128, 512], F32, tag="pd", name=f"pd_{s}")
                emit_chunks(s, po, pd, deferred)
                for part in range(3):
                    deferred.append(
                        lambda s=s, po=po, pd=pd, part=part: emit_tail(s, po, pd, part)
                    )
            for fn in deferred:
                fn()

    nc.compile()
    return nc


def _get_nc():
    global _NC
    if _NC is None:
        _NC = _build_bass()
    return _NC


def kernel(q, k, v):
    global _LAST_RESULTS
    q = np.asarray(q, dtype=np.float32)
    k = np.asarray(k, dtype=np.float32)
    v = np.asarray(v, dtype=np.float32)
    assert q.shape == (B, H, S, D)

    in_maps = []
    for c in range(N_CORES):
        h0 = HEADS_PER_CORE * c
        qTh = np.ascontiguousarray(
            q[0, h0 : h0 + 2].transpose(0, 2, 1).reshape(128, S)
        ).astype(np.float16)
        kTh = np.ascontiguousarray(
            k[0, h0 : h0 + 2].transpose(0, 2, 1).reshape(128, S)
        ).astype(np.float16)
        va = (
            v[0, h0 : h0 + 2].reshape(2, N_CHUNKS, 128, 64).transpose(0, 2, 1, 3)
        ).reshape(2, 128, 64 * N_CHUNKS).astype(ml_dtypes.bfloat16)
        va = np.ascontiguousarray(va)
        in_maps.append({"qT": qTh, "kT": kTh, "val": va})

    nc = _get_nc()
    res = run_bass_kernel_spmd(nc, in_maps, core_ids=list(range(N_CORES)))
    _LAST_RESULTS = res

    out = np.empty((B, H, S, D), dtype=np.float32)
    for c in range(N_CORES):
        ot = res.results[c]["outT"]  # [128, 2048]
        out[0, 2 * c] = ot[0:64].T
        out[0, 2 * c + 1] = ot[64:128].T
    return out


# revision 24
# speedup vs baseline: 1.0104x; 1.0104x over previous
"""Causal flash attention for Trainium2, sharded 2 heads/core over 8 cores.

Math per head: out = softmax_causal(Q K^T / sqrt(D)) @ V,  Q/K/V [S=2048, D=64] fp32.

Device layout (per core, heads h0=2c, h1=2c+1):
  qT   [128, 2048]  rows 64h+d = Q[h]^T        (D on partitions, both heads stacked)
  kT   [128, 2048]  same for K
  val  [2, 128, 1024] val[h, p, 64*kc+d] = V[h, 128*kc+p, d]
  outT [128, 2048]  rows 64h+d = out[h]^T (normalized)

Scores are computed transposed (S^T[k, q] = K_chunk @ Q^T) so no transposes are
needed anywhere.

exp is the serial bottleneck (one elem/cycle/lane on the scalar engine), so the
key chunks are split between two engines: the scalar engine computes exact exp,
and the vector engine computes a Schraudolph-style approximation (y = A*x + B
converted to int16, bitcast as bf16 = 2^(x*log2e) with mantissa-linear
interpolation, ~2% rms which softmax normalization partly cancels); the DVE
share of elements is capped to keep the end-to-end error ~1e-2.

PE work per chunk is minimized with PE array tiling:
  - QK: the two heads' matmuls occupy disjoint row halves (contraction D=64
    each) and run concurrently.
  - PV: head outputs go to col groups 0:64 / 64:128 of ONE psum bank
    (tile_position=(0,64) for h1) and run concurrently.
  - denominators: ones-vector matmuls into single psum rows {0,32,64,96} =
    (head, kc parity) of a shared D bank; a chunk pair's four matmuls occupy
    four distinct col groups and run concurrently.
po and D are single psum banks; PV/denom emission for the first chunks of a
span is deferred so the previous span's tail (which reads po/D) never
head-of-line blocks the PE FIFO.
"""

import os
import sys

import ml_dtypes
import numpy as np

sys.path.insert(0, "/opt/trn_rl_repo")

import concourse.bass as bass
import concourse.bacc as bacc
import concourse.mybir as mybir
import concourse.tile as tile
from concourse.bass_utils import run_bass_kernel_spmd

B, H, S, D = 1, 16, 2048, 64
N_CORES = 8
HEADS_PER_CORE = H // N_CORES  # 2
N_CHUNKS = S // 128  # 16 key chunks per head
N_SPANS = S // 512  # 4 query spans per head
F32 = mybir.dt.float32
R32 = mybir.dt.float32r
BF16 = mybir.dt.bfloat16
F16 = mybir.dt.float16
I16 = mybir.dt.int16

# Schraudolph constants for bf16-bit exp of (0.125 * x):
#   i16 = 0.125*x*128*log2(e) + (127*128 - C) ; bitcast bf16 ~= exp(0.125*x)
L2E = 1.4426950408889634
SCHRA_A = 0.125 * 128.0 * L2E
SCHRA_B = float(127 * 128 - 3)
DVE_EXP_SHARE = 0.0  # cap on the fraction of score elements exp'd approximately

J_DEFER = 6  # chunks whose PV/denom emission is deferred past the tail window


def _r(ap):
    return ap.bitcast(R32)

_NC = None
_LAST_RESULTS = None


def _build_bass():
    nc = bacc.Bacc("TRN2", target_bir_lowering=False)
    qT = nc.declare_dram_parameter("qT", [128, S], F16, isOutput=False)
    kT = nc.declare_dram_parameter("kT", [128, S], F16, isOutput=False)
    val = nc.declare_dram_parameter("val", [2, 128, 64 * N_CHUNKS], BF16, isOutput=False)
    outT = nc.declare_dram_parameter("outT", [128, S], F32, isOutput=True)

    total_elems = 0
    for s in range(N_SPANS):
        for kc in range(4 * s + 4):
            w = 512 * (s + 1) - max(512 * s, 128 * kc)
            total_elems += 512 + w
    dve_budget = DVE_EXP_SHARE * total_elems

    with tile.TileContext(nc) as tc:
        with (
            tc.tile_pool(name="const", bufs=1) as const,
            tc.tile_pool(name="inbuf", bufs=1) as inbuf,
            tc.tile_pool(name="pbuf", bufs=18) as pbuf,
            tc.tile_pool(name="nbuf", bufs=2) as nbuf,
            tc.tile_pool(name="ps_s", bufs=3, space="PSUM") as ps_s,
            tc.tile_pool(name="ps_po", bufs=1, space="PSUM") as ps_po,
            tc.tile_pool(name="ps_d", bufs=1, space="PSUM") as ps_d,
        ):
            ones1 = const.tile([128, 1], BF16, tag="ones1")
            nc.vector.memset(ones1, 1.0)

            # Input loads, chunked by 512 query/key columns so compute can
            # start before all DMAs land. First span processed is s=3.
            qsb = [None] * N_SPANS
            ksb = [None] * N_SPANS
            vsb = [[None] * N_SPANS, [None] * N_SPANS]
            for j in range(N_SPANS):
                qt = inbuf.tile([128, 512], F16, tag=f"q{j}", name=f"qt{j}")
                nc.sync.dma_start(out=qt, in_=qT[:, 512 * j : 512 * (j + 1)])
                qsb[j] = qt
                kt = inbuf.tile([128, 512], F16, tag=f"k{j}", name=f"kt{j}")
                nc.sync.dma_start(out=kt, in_=kT[:, 512 * j : 512 * (j + 1)])
                ksb[j] = kt
                for h in range(2):
                    vt = inbuf.tile([128, 256], BF16, tag=f"v{h}{j}", name=f"vt{h}{j}")
                    nc.sync.dma_start(out=vt, in_=val[h, :, 256 * j : 256 * (j + 1)])
                    vsb[h][j] = vt

            def k_slice(h, kc):
                return ksb[kc // 4][64 * h : 64 * h + 64, 128 * (kc % 4) : 128 * (kc % 4) + 128]

            def q_slice(h, qs, qe):
                j = qs // 512
                base = 512 * j
                return qsb[j][64 * h : 64 * h + 64, qs - base : qe - base]

            def v_slice(h, kc):
                return vsb[h][kc // 4][:, 64 * (kc % 4) : 64 * (kc % 4) + 64]

            eng_t = {"act": 0.0, "dve": 0.0, "dve_elems": 0.0}

            def emit_pv(h, kc, qs, qe, po, pe2, pv_flags):
                # col-tiled PV: h0 -> partitions 0:64, h1 -> 64:128 of the
                # same bank; the two matmuls run concurrently.
                qb = max(qs, 128 * kc)
                w = qe - qb
                # start/stop are per psum col-group region: h0 and h1 write
                # disjoint partition ranges, each needs its own has_written
                # clear (and the first emitted per head must be full-width).
                start = pv_flags[h] == 0
                stop = pv_flags[h] == pv_flags["total"] // 2 - 1
                pv_flags[h] += 1
                if start:
                    assert w == 512, f"first PV of head {h} must be full-width"
                pv_flags["n"] += 1
                nc.tensor.matmul(
                    po[64 * h : 64 * h + 64, qb - qs : qb - qs + w],
                    v_slice(h, kc),
                    pe2[:, 512 * h : 512 * h + w],
                    start=start,
                    stop=stop,
                    tile_position=(0, 64 * h),
                )

            def emit_den(h, kc, qs, qe, pd, pe2, den_flags, s):
                # denominator partial: ones-vector matmul into psum row
                # 64*h + 32*(kc&1); a chunk pair's four matmuls use four
                # distinct col groups and run concurrently. Span 0 has no
                # full-width chunk of odd parity, so it uses one group per
                # head (first emission must be full-width: start=True only
                # clears has_written for the region it writes).
                qb = max(qs, 128 * kc)
                w = qe - qb
                g = 64 * h + (32 * (kc & 1) if s > 0 else 0)
                start = den_flags[g] == 0
                stop = den_flags[g] == den_flags[g, "total"] - 1
                den_flags[g] += 1
                if start:
                    assert w == 512, f"first den of group {g} must be full (s={s} kc={kc})"
                nc.tensor.matmul(
                    pd[g : g + 1, qb - qs : qb - qs + w],
                    ones1,
                    pe2[:, 512 * h : 512 * h + w],
                    start=start,
                    stop=stop,
                    tile_position=(0, g),
                )

            class Flags(dict):
                def __missing__(self, k):
                    return 0

            def emit_chunks(s, po, pd, deferred, first_span):
                qs, qe = 512 * s, 512 * (s + 1)
                nkc = 4 * s + 4
                # PV/den emission for the first J chunks is deferred until
                # the previous span's tail has freed po/pd (the tail mul is
                # popped at kc==3 and completes by ~kc 5); deferred PVs are
                # then emitted one per chunk (catch-up) to avoid a PE burst.
                # The first emitted PV/den per psum region must be a
                # full-width chunk (start=True clears has_written only for
                # the region it writes), hence J <= 4s.
                J = 0 if first_span else min(J_DEFER, 4 * s)
                pe2s = []
                pv_flags = Flags()
                pv_flags["total"] = 2 * nkc
                den_flags = Flags()
                for h in range(2):
                    if s == 0:
                        den_flags[64 * h, "total"] = nkc
                    else:
                        for par in range(2):
                            den_flags[64 * h + 32 * par, "total"] = sum(
                                1 for kc in range(nkc) if (kc & 1) == par
                            )
                den_pairs_done = 0
                for kc in range(nkc):
                    # pop the previous span's tail pieces a few chunks in so
                    # their long-latency chains never head-of-line block.
                    if kc in (0, 1, 3) and deferred:
                        deferred.pop(0)()
                    qb = max(qs, 128 * kc)
                    w = qe - qb
                    pg = ps_s.tile([128, 1024], F32, tag="pss", name=f"pg_{s}_{kc}")
                    pe2 = pbuf.tile([128, 1024], BF16, tag="pe", name=f"pe_{s}_{kc}")
                    pe2s.append(pe2)
                    for h in range(2):
                        nc.tensor.matmul(
                            pg[:, 512 * h : 512 * h + w],
                            k_slice(h, kc),
                            q_slice(h, qb, qe),
                            start=True,
                            stop=True,
                        )
                    fd = 512 + w
                    t_act = (fd + 204) / 1.2 + 232
                    t_dve = (fd + 120) / 0.96 + 275
                    use_dve = (
                        eng_t["dve"] + t_dve <= eng_t["act"] + t_act
                        and eng_t["dve_elems"] + fd <= dve_budget
                    )
                    if use_dve:
                        eng_t["dve"] += t_dve
                        eng_t["dve_elems"] += fd
                        nc.vector.tensor_scalar(
                            out=pe2.bitcast(I16)[:, :fd],
                            in0=pg[:, :fd],
                            scalar1=SCHRA_A,
                            scalar2=SCHRA_B,
                            op0=mybir.AluOpType.mult,
                            op1=mybir.AluOpType.add,
                        )
                    else:
                        eng_t["act"] += t_act
                        nc.scalar.activation(
                            out=pe2[:, :fd],
                            in_=pg[:, :fd],
                            func=mybir.ActivationFunctionType.Exp,
                            scale=0.125,
                        )
                    if kc >= 4 * s:
                        # diagonal chunk: zero the strict upper triangle of
                        # the first 128 cols, after exp, on gpsimd.
                        for h in range(2):
                            nc.gpsimd.affine_select(
                                out=pe2[:, 512 * h : 512 * h + 128],
                                in_=pe2[:, 512 * h : 512 * h + 128],
                                compare_op=mybir.AluOpType.is_ge,
                                fill=0.0,
                                base=0,
                                pattern=[[1, 128]],
                                channel_multiplier=-1,
                            )
                    if kc >= J:
                        for h in range(2):
                            emit_pv(h, kc, qs, qe, po, pe2, pv_flags)
                        # catch-up: one deferred PV per chunk
                        cc = kc - J
                        if J and cc < J:
                            for h in range(2):
                                emit_pv(h, cc, qs, qe, po, pe2s[cc], pv_flags)
                    if s == 0:
                        # span 0: one den group per head, emit immediately
                        if kc >= J:
                            for h in range(2):
                                emit_den(h, kc, qs, qe, pd, pe2s[kc], den_flags, s)
                    elif kc & 1 and kc >= 3:
                        # den pairs lag 3 chunks behind (pd WAR window);
                        # a pair's four matmuls use 4 col groups concurrently
                        p0 = 2 * den_pairs_done
                        if p0 + 1 <= kc - 2:
                            den_pairs_done += 1
                            for cc in (p0, p0 + 1):
                                for h in range(2):
                                    emit_den(h, cc, qs, qe, pd, pe2s[cc], den_flags, s)
                # catch-up covers all deferred PVs because nkc >= 2J for
                # every span; only the last den pair remains to flush.
                if s > 0:
                    while den_pairs_done * 2 < nkc:
                        p0 = 2 * den_pairs_done
                        den_pairs_done += 1
                        for cc in (p0, p0 + 1):
                            for h in range(2):
                                emit_den(h, cc, qs, qe, pd, pe2s[cc], den_flags, s)
                assert pv_flags["n"] == pv_flags["total"], (
                    pv_flags["n"], pv_flags["total"], s, J
                )

            def emit_tail(s, po, pd, part):
                qs, qe = 512 * s, 512 * (s + 1)
                if part == 0:
                    # D bank -> SBUF (one FD-bound DVE copy covers all rows)
                    den_sb = nbuf.tile([97, 512], F32, tag="den")
                    nc.vector.tensor_copy(out=den_sb, in_=pd[0:97, :])
                    # gather the partial rows onto 128 partitions so
                    # reciprocal runs at FD=4 per group; parallel queues.
                    dsum = nbuf.tile([128, 16], F32, tag="dsum")
                    groups = (0, 32, 64, 96) if s > 0 else (0, 64)
                    qq = [nc.sync, nc.gpsimd, nc.sync, nc.gpsimd]
                    for i, g in enumerate(groups):
                        qq[i % len(qq)].dma_start(
                            out=dsum[:, 4 * i : 4 * i + 4],
                            in_=den_sb[g : g + 1, :],
                        )
                    state["dsum"] = dsum
                elif part == 1:
                    dsum = state["dsum"]
                    d8 = nbuf.tile([128, 8], F32, tag="d8")
                    if s > 0:
                        nc.vector.tensor_add(
                            out=d8[:, 0:4], in0=dsum[:, 0:4], in1=dsum[:, 4:8]
                        )
                        nc.vector.tensor_add(
                            out=d8[:, 4:8], in0=dsum[:, 8:12], in1=dsum[:, 12:16]
                        )
                    else:
                        nc.vector.tensor_copy(out=d8, in_=dsum[:, 0:8])
                    r8 = nbuf.tile([128, 8], F32, tag="r8")
                    nc.vector.reciprocal(out=r8, in_=d8)
                    r2 = nbuf.tile([2, 512], F32, tag="r2")
                    for h in range(2):
                        (nc.sync if h == 0 else nc.gpsimd).dma_start(
                            out=r2[h : h + 1, :],
                            in_=r8[:, 4 * h : 4 * h + 4],
                        )
                    rb = nbuf.tile([128, 512], F32, tag="rb")
                    for h in range(2):
                        (nc.sync if h == 0 else nc.gpsimd).dma_start(
                            out=rb[64 * h : 64 * h + 64, :],
                            in_=r2[h : h + 1, :].unsqueeze(1).broadcast_to(
                                [1, 64, 512]
                            ),
                        )
                    state["rb"] = rb
                else:
                    rb = state["rb"]
                    o2 = nbuf.tile([128, 512], F32, tag="o2")
                    nc.vector.tensor_mul(out=o2, in0=po[0:128, :], in1=rb)
                    eng_t["dve"] += 890
                    nc.sync.dma_start(out=outT[:, qs:qe], in_=o2)

            state = {}
            deferred = []
            for s in (0, 1, 2, 3):
                po = ps_po.tile([128, 512], F32, tag="po", name=f"po_{s}")
                pd = ps_d.tile([128, 512], F32, tag="pd", name=f"pd_{s}")
                emit_chunks(s, po, pd, deferred, first_span=(s == 0))
                for part in range(3):
                    deferred.append(
                        lambda s=s, po=po, pd=pd, part=part: emit_tail(s, po, pd, part)
                    )
            for fn in deferred:
                fn()

    nc.compile()
    return nc


def _get_nc():
    global _NC
    if _NC is None:
        _NC = _build_bass()
    return _NC


def kernel(q, k, v):
    global _LAST_RESULTS
    q = np.asarray(q, dtype=np.float32)
    k = np.asarray(k, dtype=np.float32)
    v = np.asarray(v, dtype=np.float32)
    assert q.shape == (B, H, S, D)

    in_maps = []
    for c in range(N_CORES):
        h0 = HEADS_PER_CORE * c
        qTh = np.ascontiguousarray(
            q[0, h0 : h0 + 2].transpose(0, 2, 1).reshape(128, S)
        ).astype(np.float16)
        kTh = np.ascontiguousarray(
            k[0, h0 : h0 + 2].transpose(0, 2, 1).reshape(128, S)
        ).astype(np.float16)
        va = (
            v[0, h0 : h0 + 2].reshape(2, N_CHUNKS, 128, 64).transpose(0, 2, 1, 3)
        ).reshape(2, 128, 64 * N_CHUNKS).astype(ml_dtypes.bfloat16)
        va = np.ascontiguousarray(va)
        in_maps.append({"qT": qTh, "kT": kTh, "val": va})

    nc = _get_nc()
    res = run_bass_kernel_spmd(nc, in_maps, core_ids=list(range(N_CORES)))
    _LAST_RESULTS = res

    out = np.empty((B, H, S, D), dtype=np.float32)
    for c in range(N_CORES):
        ot = res.results[c]["outT"]  # [128, 2048]
        out[0, 2 * c] = ot[0:64].T
        out[0, 2 * c + 1] = ot[64:128].T
    return out
